# revision 15
# baseline (speedup 1.0000x reference)
"""TRN2 Bass/Tile kernel for nn_Loss_58317065945194.

Loss: per-sample EMD with r=2 over C=10 channels:
    d = p - q                       # [B, C]
    S = cumsum(d, axis=1)           # per-sample prefix sums
    per_sample = sqrt(mean(S**2))   # [B]
    out = mean(per_sample)          # scalar

Strategy (pure data parallel, 8 cores), TensorE-centric:
  - Shard B across 8 cores (262144 samples/core). Host quantizes p, q
    to fp8-e4m3 (TRN FP8_EXP4-compatible for |x|<=240) and lays each
    core's shard out channel-major: 12 samples per SBUF column across
    120 partitions (partition k = 10*block + channel), F=22528 columns
    = 11 chunks x 2048. p and q are interleaved as the two k-subtiles
    of a DoubleRow fp8 matmul (K_eff = 240).
  - mm1 (TensorE, DoubleRow): stationary [120,2,128] holding
    (+L^T, -L^T) block-diagonal (12 blocks of the 10x10 triangular
    ones) computes S = L p - L q for 12 samples/column directly into
    PSUM [128, 1024] (2 banks per pair of 512-col matmuls). Replaces
    the Vector-engine scan + boundary fixup of the old design.
  - drain: square PSUM->SBUF fp8. Per [120,1024] pair either
      Act:  one activation(Square)
      DVE:  tensor_copy to fp16 then tensor_tensor mult -> fp8
      GP :  DVE copy to fp16, GpSimd does the multiply
    (assignment pattern tuned for engine balance).
  - mm3 (TensorE, plain fp8 + 4x column tiling; DoubleRow is
    ISA-illegal with col_grp != 0xf): stationary block-ones [120,32]
    sums each sample's 10 squared prefix sums per 512-col bank; eight
    mm3 outputs pack one PSUM tile [128,1024] at partition offsets
    0/32/64/96 x 2 column halves. Replaces the DVE tensor_reduce.
  - Act sqrt(scale=0.1) on each packed bank with accum_out -> per-core
    partial sums of per-sample losses; host sums partials and divides
    by B.
"""

import sys

import numpy as np

if "/opt/trn_rl_repo" not in sys.path:
    sys.path.insert(0, "/opt/trn_rl_repo")

import ml_dtypes

N_CORES = 8
B, C = 2097152, 10
BS = B // N_CORES            # samples per core shard (262144)
SPC = 12                     # samples per column (10*12=120 partitions)
P = SPC * C                  # 120 active partitions
CHUNKS = [1024, 1024] + [4096] * 5   # column count per DMA chunk
F = sum(CHUNKS)              # 22528 columns (270336 sample slots, zero-padded)
SPAD = F * SPC               # padded samples per core
NPAIR = F // 1024            # 22 pairs of 512-col banks
NBANK = 2 * NPAIR            # 44 drained 512-col banks -> 44 mm3 slots
NSQ = (NBANK + 7) // 8       # 6 sqrt groups of 8 mm3 slots each
NWARM = 10                   # dummy matmuls to pre-warm the PE HAM clock
# drain engine per pair: A=Act square, D=DVE copy+square, G=DVE copy+GP square
PATTERN = ["A", "D", "A", "G"]

_cache = {}


def _build_program():
    import concourse.tile as tile
    from concourse import bacc, mybir

    f32, f16, f8 = mybir.dt.float32, mybir.dt.float16, mybir.dt.float8e4
    Alu = mybir.AluOpType
    Act = mybir.ActivationFunctionType
    DR = mybir.MatmulPerfMode.DoubleRow

    nc = bacc.Bacc(
        "TRN2", target_bir_lowering=False, debug=False, num_devices=N_CORES
    )
    pq_d = nc.dram_tensor("pq", [P, 2 * F], f8, kind="ExternalInput").ap()
    w1_d = nc.dram_tensor("w1", [P, 2 * 128], f8, kind="ExternalInput").ap()
    w2_d = nc.dram_tensor("w2", [P, 32], f8, kind="ExternalInput").ap()
    o_d = nc.dram_tensor("partial", [128, NSQ], f32, kind="ExternalOutput").ap()

    with tile.TileContext(nc) as tc:
        with (
            tc.tile_pool(name="const", bufs=1) as const,
            tc.tile_pool(name="io", bufs=3) as io,
            tc.tile_pool(name="sq", bufs=6) as sqp,
            tc.tile_pool(name="tmp", bufs=4) as tmp,
            tc.tile_pool(name="dump", bufs=2) as dump,
            tc.tile_pool(name="accp", bufs=1) as accp,
            tc.psum_pool(name="psS", bufs=2) as psS,
            tc.psum_pool(name="psU", bufs=2) as psU,
        ):
            w1 = const.tile([P, 2, 128], f8)
            w2 = const.tile([P, 32], f8)
            wd = const.tile([P, 2, 128], f8)
            wr = const.tile([P, 2, 512], f8)
            nc.gpsimd.memset(wd[:], 0.0)
            nc.gpsimd.memset(wr[:], 0.0)
            acc = accp.tile([128, NSQ], f32)
            nc.gpsimd.memset(acc[:], 0.0)

            # queue the input chunks first so the first tile lands ASAP;
            # w1/w2 are only needed once the first matmul issues
            pq_tiles = []
            col0 = 0
            for ci, cw in enumerate(CHUNKS):
                pq_t = io.tile([P, 2, cw], f8, tag=f"pq{cw}", name=f"pq{ci}")
                nc.sync.dma_start(pq_t[:], pq_d[:, 2 * col0 : 2 * (col0 + cw)])
                pq_tiles.append((pq_t, cw))
                col0 += cw
                if ci == 0:
                    nc.sync.dma_start(w1[:], w1_d[:])
                    nc.sync.dma_start(w2[:], w2_d[:])
                    # pre-warm the PE HAM clock while the first chunk is in
                    # flight: dummy matmuls on the zeroed weight tile
                    Sw = psS.tile([128, 1024], f32, tag="S2", name="Swarm")
                    for k in range(NWARM):
                        nc.tensor.matmul(
                            Sw[:, 0:512],
                            wd[:],
                            wr[:],
                            start=True,
                            stop=True,
                            perf_mode=DR,
                        )

            sq_q = []      # pending (pair_idx, sq_tile) awaiting mm3
            ss_tiles = {}  # sqrt group -> [128, 1024] psum tile (8 mm3 slots)

            def issue_mm3s(pi, sq_t):
                for half in range(2):
                    bi = 2 * pi + half            # global 512-col bank index
                    grp, slot = bi // 8, bi % 8
                    if slot == 0:
                        ss_tiles[grp] = psU.tile(
                            [128, 1024], f32, tag="ss", name=f"ss{grp}"
                        )
                    ss = ss_tiles[grp]
                    cp, ch = slot % 4, slot // 4  # col-group, column half
                    nc.tensor.matmul(
                        ss[32 * cp : 32 * cp + 32, 512 * ch : 512 * ch + 512],
                        w2[:],
                        sq_t[:, 512 * half : 512 * half + 512],
                        start=True,
                        stop=True,
                        tile_position=(0, 32 * cp),
                    )
                    if bi == NBANK - 1 or slot == 7:
                        cols = 512 * (ch + 1)
                        dmp = dump.tile([128, 1024], f16, tag="dmp")
                        nc.scalar.activation(
                            dmp[:, 0:cols],
                            ss[:, 0:cols],
                            Act.Sqrt,
                            scale=1.0 / C,
                            accum_out=acc[:, grp : grp + 1],
                        )
                        del ss_tiles[grp]

            pi = 0
            for pq_t, cw in pq_tiles:
                for h in range(cw // 1024):
                    S2 = psS.tile([128, 1024], f32, tag="S2")
                    for j in range(2):
                        nc.tensor.matmul(
                            S2[:, 512 * j : 512 * (j + 1)],
                            w1[:],
                            pq_t[:, :, (2 * h + j) * 512 : (2 * h + j + 1) * 512],
                            start=True,
                            stop=True,
                            perf_mode=DR,
                        )
                    sq_t = sqp.tile([P, 1024], f8, tag="sq")
                    eng = PATTERN[pi % len(PATTERN)]
                    if eng == "A":
                        nc.scalar.activation(sq_t[:], S2[0:P, :], Act.Square)
                    else:
                        s16 = tmp.tile([P, 1024], f16, tag="s16")
                        nc.vector.tensor_copy(s16[:], S2[0:P, :])
                        e = nc.vector if eng == "D" else nc.gpsimd
                        e.tensor_tensor(sq_t[:], s16[:], s16[:], Alu.mult)
                    sq_q.append((pi, sq_t))
                    # lagged mm3 to keep PE fed while drains complete; drop
                    # the lag near the end to shorten the tail
                    lag = 3 if pi < NPAIR - 3 else 0
                    while len(sq_q) > lag:
                        issue_mm3s(*sq_q.pop(0))
                    pi += 1
            while sq_q:
                issue_mm3s(*sq_q.pop(0))
            nc.sync.dma_start(o_d[:], acc[:])
    nc.compile()
    return nc


def _tri_blocks():
    """W1 [P, 2, 128]: (+L^T | -L^T) block-diagonal; W2 [P, 32] block ones."""
    f8 = ml_dtypes.float8_e4m3
    w1 = np.zeros((P, 2, 128), np.float32)
    w2 = np.zeros((P, 32), np.float32)
    for b in range(SPC):
        for cs in range(C):          # source channel (k = 10b+cs)
            for co in range(C):      # output channel (m = 10b+co)
                if cs <= co:
                    w1[10 * b + cs, 0, 10 * b + co] = 1.0
                    w1[10 * b + cs, 1, 10 * b + co] = -1.0
            w2[10 * b + cs, b] = 1.0
    return w1.astype(f8), w2.astype(f8)


def _make_in_maps(p, q):
    f8 = ml_dtypes.float8_e4m3
    p = np.asarray(p, dtype=np.float32).reshape(B, C)
    q = np.asarray(q, dtype=np.float32).reshape(B, C)
    w1, w2 = _tri_blocks()
    w1 = np.ascontiguousarray(w1.reshape(P, 256))

    def lay(x):
        # [BS, C] -> padded [F, SPC, C] -> [SPC, C, F] = [120, F]
        xp = np.zeros((SPAD, C), np.float32)
        xp[:BS] = x
        return xp.reshape(F, SPC, C).transpose(1, 2, 0).reshape(P, F)

    in_maps = []
    for r in range(N_CORES):
        pp = lay(p[r * BS : (r + 1) * BS])
        qq = lay(q[r * BS : (r + 1) * BS])
        pq = np.empty((P, 2 * F), np.float32)
        col0 = 0
        for cw in CHUNKS:
            pq[:, 2 * col0 : 2 * col0 + cw] = pp[:, col0 : col0 + cw]
            pq[:, 2 * col0 + cw : 2 * (col0 + cw)] = qq[:, col0 : col0 + cw]
            col0 += cw
        in_maps.append({"pq": pq.astype(f8), "w1": w1, "w2": w2})
    return in_maps


def kernel(p, q, r):
    assert int(r) == 2, f"kernel specialized for r=2, got {r}"
    if "nc" not in _cache:
        _cache["nc"] = _build_program()
    nc = _cache["nc"]

    in_maps = _make_in_maps(p, q)

    from concourse.bass_utils import run_bass_kernel_spmd

    res = run_bass_kernel_spmd(nc, in_maps, list(range(N_CORES)))
    total = 0.0
    for r_ in res.results:
        total += r_["partial"].astype(np.float64).sum()
    return np.float32(total / B)
